# revision 15
# baseline (speedup 1.0000x reference)
"""AKDN (KG-attention + LightGCN + BPR) distributed Bass kernel for 8 TRN2 cores.

Strategy (hardcoded from spec sharding_hint):
- Items/users sharded by destination across 8 cores (degree-sorted, round-robin
  dealt so shards and windows are degree-balanced). All index remapping is done
  host-side (integer work only); all float compute runs on device.
- Edge segment-sums: per dest-window (128 dests on partitions) padded slot
  grids, gathered with gpsimd.dma_gather from replicated tables split into
  int16-addressable ranges (4 user ranges / 2 item ranges). Pad slots gather
  row 0 and are killed by norm=0. Reduce over slots on DVE.
- KG attention: entity rows compacted once (stage A, 8 entity ranges) into a
  per-core VTMP in chunk-contiguous order; each layer stage-B gathers from the
  <=32K-row chunk with a single range. score uses the algebraic folding
  score = sum_d hv * (R@Weff)[rel] + (R@b)[rel]   (kills the per-item matmul).
- Gate fusion per window via PE transposes + matmuls, sigmoid on ACT.
- Cross-core: AllGather of u1/i1/i2 tables; BPR partials AllReduce.
"""

import numpy as np
import sys

sys.path.insert(0, "/opt/trn_rl_repo")

NU, NI, NE, NR = 100000, 50000, 200000, 64
D, K, EDGES, LAYERS, B = 64, 16, 2000000, 2, 4096
REG = 1e-4
M = 8
IPC = 6272      # items/core (49*128)
UPC = 12544     # users/core (98*128)
WI = IPC // 128  # 49
WU = UPC // 128  # 98
NIP = M * IPC   # 50176
NUP = M * UPC   # 100352
NRU = 4         # user ranges
NRI = 2         # item ranges
RSU = NUP // NRU   # 25088
RSI = NIP // NRI   # 25088
NRE = 8
RSE = NE // NRE    # 25000
VCH = 8            # windows per v-chunk
NCH = (WI + VCH - 1) // VCH  # 7
CHSL = VCH * 128 * K         # 16384 slots/chunk

FP32 = None  # filled on import of mybir inside build


def _deal_perm(deg, n_orig, per_core, ncores=M):
    """Sort by degree desc, deal round-robin to cores. Returns new_index[orig]
    (global new id = core*per_core + local)."""
    order = np.argsort(-deg, kind="stable")
    j = np.arange(n_orig)
    core = j % ncores
    local = j // ncores
    newid = np.empty(n_orig, dtype=np.int64)
    newid[order] = core * per_core + local
    return newid


def _wrap16(idx_stream):
    """[N] -> [128, N//16] int16 wrapped (pos i -> row i%16, col i//16) and
    replicated across the 8 groups of 16 partitions."""
    n = idx_stream.shape[0]
    assert n % 16 == 0
    w = idx_stream.reshape(n // 16, 16).T.astype(np.int16)  # [16, n//16]
    return np.tile(w, (8, 1))  # [128, n//16]


def _edge_side(dst_new, src_new, norms, per_core, nw, nranges, rsize):
    """Pack one segment-sum side. Returns per-(w,r) J table (shared across
    cores) and per-core idx/norm arrays.

    dst_new/src_new: permuted-global dest & source ids per edge.
    """
    core = dst_new // per_core
    ploc = dst_new % per_core
    w = ploc // 128
    p = ploc % 128
    r = src_new // rsize
    sloc = (src_new % rsize).astype(np.int64)

    # counts[core, w, r, p]
    key = ((core * nw + w) * nranges + r) * 128 + p
    counts = np.bincount(key, minlength=M * nw * nranges * 128).reshape(
        M, nw, nranges, 128
    )
    J = counts.max(axis=(0, 3))  # [nw, nranges]
    J = np.maximum(J, 1)

    # slot index j within (core,w,r,p) group
    order = np.argsort(key, kind="stable")
    sk = key[order]
    starts = np.r_[0, np.cumsum(np.bincount(sk, minlength=key.max() + 1))][:-1]
    jslot = np.arange(len(sk)) - starts[sk]

    # column offsets: per window w, ranges laid out consecutively; windows
    # consecutive. colbase[w, r] (in J-columns), grid position base in slots.
    colbase = np.zeros((nw, nranges), dtype=np.int64)
    run = 0
    for wi in range(nw):
        for ri in range(nranges):
            colbase[wi, ri] = run
            run += J[wi, ri]
    totj = run  # total columns
    tot = totj * 128  # total slots

    idxs = np.zeros((M, tot), dtype=np.int64)
    nrms = np.zeros((M, 128, totj), dtype=np.float32)
    oc = core[order]
    ow = w[order]
    orr = r[order]
    op = p[order]
    os_ = sloc[order]
    on = norms[order]
    col = colbase[ow, orr] + jslot
    pos = col * 128 + op
    idxs[oc, pos] = os_
    nrms[oc, op, col] = on

    eidx = np.stack([_wrap16(idxs[c]) for c in range(M)])  # [M,128,tot//16]
    return J, colbase, totj, eidx, nrms


def _prep(inputs):
    """All host-side integer/layout preprocessing. Returns (meta, per-core
    input maps)."""
    ei = inputs["edge_i"].astype(np.int64)
    eu = inputs["edge_u"].astype(np.int64)
    norm = inputs["edge_norm"].astype(np.float32)

    deg_i = np.bincount(ei, minlength=NI)
    deg_u = np.bincount(eu, minlength=NU)
    inew = _deal_perm(deg_i, NI, IPC)   # [NI] -> global new id
    unew = _deal_perm(deg_u, NU, UPC)

    # permuted padded tables (row placement only)
    U0 = np.zeros((NUP, D), dtype=np.float32)
    U0[unew] = inputs["user_emb_w"]
    I0 = np.zeros((NIP, D), dtype=np.float32)
    I0[inew] = inputs["item_emb_w"]

    # ---- edge sides ----
    JI, cbI, totjI, eidx_i, enrm_i = _edge_side(
        inew[ei], unew[eu], norm, IPC, WI, NRU, RSU
    )
    JU, cbU, totjU, eidx_u, enrm_u = _edge_side(
        unew[eu], inew[ei], norm, UPC, WU, NRI, RSI
    )

    # ---- KG: per-core item-window slot grids ----
    kg_ent = inputs["kg_ent"].astype(np.int64)  # [NI, K]
    kg_rel = inputs["kg_rel"].astype(np.int64)
    # per new item id: original item (for padded: none -> entity 0 / rel 0)
    orig_of_new = np.zeros(NIP, dtype=np.int64)
    has = np.zeros(NIP, dtype=bool)
    orig_of_new[inew] = np.arange(NI)
    has[inew] = True

    ent_g = np.zeros((M, IPC, K), dtype=np.int64)
    rel_g = np.zeros((M, IPC, K), dtype=np.int64)
    for c in range(M):
        sl = slice(c * IPC, (c + 1) * IPC)
        o = orig_of_new[sl]
        h = has[sl]
        e = np.where(h[:, None], kg_ent[o], 0)
        r_ = np.where(h[:, None], kg_rel[o], 0)
        # sort each item's neighbors by entity id (stage-A range runs)
        srt = np.argsort(e, axis=1, kind="stable")
        ent_g[c] = np.take_along_axis(e, srt, axis=1)
        rel_g[c] = np.take_along_axis(r_, srt, axis=1)

    # stage-A: per (chunk, range) compact runs. counts -> shared padded counts
    er = ent_g // RSE                     # range per slot [M, IPC, K]
    chunk = (np.arange(IPC) // 128) // VCH  # [IPC]
    cntA = np.zeros((M, NCH, NRE), dtype=np.int64)
    for c in range(M):
        keyA = chunk[:, None] * NRE + er[c]
        cntA[c] = np.bincount(keyA.ravel(), minlength=NCH * NRE).reshape(NCH, NRE)
    PCA = ((cntA.max(axis=0) + 127) // 128) * 128  # [NCH, NRE] padded counts

    # vtmp layout: chunk base = sum of previous chunks' padded totals
    chtot = PCA.sum(axis=1)  # [NCH]
    chbase = np.r_[0, np.cumsum(chtot)][:-1]
    vtmp_rows = int(chtot.sum())

    vidxa = np.zeros((M, PCA.sum()), dtype=np.int64)
    vposb = np.zeros((M, IPC, K), dtype=np.int64)  # vtmp pos of each slot
    runbase = np.zeros((NCH, NRE), dtype=np.int64)
    run = 0
    for ci in range(NCH):
        for ri in range(NRE):
            runbase[ci, ri] = run
            run += PCA[ci, ri]
    for c in range(M):
        eflat = ent_g[c].ravel()
        cflat = np.repeat(chunk, K)
        rflat = eflat // RSE
        keyA = cflat * NRE + rflat
        order = np.argsort(keyA, kind="stable")
        sk = keyA[order]
        starts = np.r_[0, np.cumsum(np.bincount(sk, minlength=NCH * NRE))][:-1]
        jj = np.arange(len(sk)) - starts[sk]
        posA = runbase[cflat[order], rflat[order]] + jj
        vidxa[c, posA] = eflat[order] % RSE
        vposb_flat = np.empty(IPC * K, dtype=np.int64)
        vposb_flat[order] = posA
        vposb[c] = vposb_flat.reshape(IPC, K)

    vidxa_w = np.stack([_wrap16(vidxa[c]) for c in range(M)])

    # stage-B idx: per window [128*K] positions: pos j*128+p -> slot (item p, k j)
    vidxb = np.zeros((M, IPC * K), dtype=np.int64)
    relidx = np.zeros((M, IPC * K), dtype=np.int64)
    for c in range(M):
        vb = vposb[c].reshape(WI, 128, K)
        rl = rel_g[c].reshape(WI, 128, K)
        for wi in range(WI):
            ci = wi // VCH
            blk = vb[wi] - chbase[ci]          # [128, K] local to chunk
            assert blk.min() >= 0 and blk.max() < 32768
            vidxb[c, wi * 128 * K: (wi + 1) * 128 * K] = blk.T.ravel()
            relidx[c, wi * 128 * K: (wi + 1) * 128 * K] = rl[wi].T.ravel()
    vidxb_w = np.stack([_wrap16(vidxb[c]) for c in range(M)])
    relidx_w = np.stack([_wrap16(relidx[c]) for c in range(M)])

    # ---- BPR batch ----
    bu_new = unew[inputs["user"].astype(np.int64)]
    bp_new = inew[inputs["pos_item"].astype(np.int64)]
    bn_new = inew[inputs["neg_item"].astype(np.int64)]
    bcore = bu_new // UPC
    cnt = np.bincount(bcore, minlength=M)
    BM = int(((cnt.max() + 127) // 128) * 128)
    bu = np.zeros((M, BM), dtype=np.int32)
    bp = np.zeros((M, BM), dtype=np.int32)
    bn = np.zeros((M, BM), dtype=np.int32)
    bw = np.zeros((M, BM), dtype=np.float32)
    for c in range(M):
        sel = np.nonzero(bcore == c)[0]
        n = len(sel)
        bu[c, :n] = (bu_new[sel] % UPC).astype(np.int32)
        bp[c, :n] = bp_new[sel].astype(np.int32)
        bn[c, :n] = bn_new[sel].astype(np.int32)
        bw[c, :n] = 1.0

    meta = dict(
        JI=JI, cbI=cbI, totjI=totjI,
        JU=JU, cbU=cbU, totjU=totjU,
        PCA=PCA, runbase=runbase, chbase=chbase, vtmp_rows=vtmp_rows,
        BM=BM,
    )

    # weight layout-only transforms
    wk = inputs["Wk_w"].reshape(2 * D, 2 * D).astype(np.float32)  # [128,128]
    wkbT = inputs["Wk_b"].T.astype(np.float32).copy()             # [64,2]
    waT = np.concatenate(
        [inputs["Wa_w"][l].T for l in range(2)], axis=0
    ).astype(np.float32)  # [128,64] rows 64l:64l+64 = Wa_w[l].T (= [d,o])
    wbT = np.concatenate(
        [inputs["Wb_w"][l].T for l in range(2)], axis=0
    ).astype(np.float32)
    wabT = inputs["Wa_b"].T.astype(np.float32).copy()  # [64,2]
    wbbT = inputs["Wb_b"].T.astype(np.float32).copy()

    in_maps = []
    for c in range(M):
        in_maps.append({
            "u0": U0, "i0": I0,
            "ent": inputs["entity_emb_w"].astype(np.float32),
            "relw": inputs["relation_emb_w"].astype(np.float32),
            "wk": wk, "wkbT": wkbT, "waT": waT, "wbT": wbT,
            "wabT": wabT, "wbbT": wbbT,
            "ishard0": np.ascontiguousarray(I0[c * IPC:(c + 1) * IPC]),
            "ushard0": np.ascontiguousarray(U0[c * UPC:(c + 1) * UPC]),
            "eidx_i": eidx_i[c], "enrm_i": enrm_i[c],
            "eidx_u": eidx_u[c], "enrm_u": enrm_u[c],
            "vidxa": vidxa_w[c], "vidxb": vidxb_w[c], "relidx": relidx_w[c],
            "bu": bu[c].reshape(BM // 128, 128).T.copy(),
            "bp": bp[c].reshape(BM // 128, 128).T.copy(),
            "bn": bn[c].reshape(BM // 128, 128).T.copy(),
            "bw": bw[c].reshape(BM // 128, 128).T.copy(),
        })
    return meta, in_maps


def _build(meta):
    import os
    PH = int(os.environ.get("AKDN_PHASE", "9"))
    import concourse.bass as bass
    import concourse.mybir as mybir
    import concourse.tile as tile
    from concourse import bacc
    from concourse.masks import make_identity

    f32 = mybir.dt.float32
    i16 = mybir.dt.int16
    i32 = mybir.dt.int32
    ALU = mybir.AluOpType
    ACTF = mybir.ActivationFunctionType

    JI, cbI, totjI = meta["JI"], meta["cbI"], meta["totjI"]
    JU, cbU, totjU = meta["JU"], meta["cbU"], meta["totjU"]
    PCA, runbase, chbase = meta["PCA"], meta["runbase"], meta["chbase"]
    BM = meta["BM"]
    BC = BM // 128

    nc = bacc.Bacc(None, target_bir_lowering=False, debug=False)

    # ---- I/O ----
    u0 = nc.dram_tensor("u0", [NUP, D], f32, kind="ExternalInput")
    i0 = nc.dram_tensor("i0", [NIP, D], f32, kind="ExternalInput")
    ent = nc.dram_tensor("ent", [NE, D], f32, kind="ExternalInput")
    relw = nc.dram_tensor("relw", [NR, D], f32, kind="ExternalInput")
    wk = nc.dram_tensor("wk", [128, 128], f32, kind="ExternalInput")
    wkbT = nc.dram_tensor("wkbT", [64, 2], f32, kind="ExternalInput")
    waT = nc.dram_tensor("waT", [128, 64], f32, kind="ExternalInput")
    wbT = nc.dram_tensor("wbT", [128, 64], f32, kind="ExternalInput")
    wabT = nc.dram_tensor("wabT", [64, 2], f32, kind="ExternalInput")
    wbbT = nc.dram_tensor("wbbT", [64, 2], f32, kind="ExternalInput")
    ishard0 = nc.dram_tensor("ishard0", [IPC, D], f32, kind="ExternalInput")
    ushard0 = nc.dram_tensor("ushard0", [UPC, D], f32, kind="ExternalInput")
    eidx_i_d = nc.dram_tensor("eidx_i", [128, totjI * 8], i16, kind="ExternalInput")
    enrm_i_d = nc.dram_tensor("enrm_i", [128, totjI], f32, kind="ExternalInput")
    eidx_u_d = nc.dram_tensor("eidx_u", [128, totjU * 8], i16, kind="ExternalInput")
    enrm_u_d = nc.dram_tensor("enrm_u", [128, totjU], f32, kind="ExternalInput")
    ATOT = int(PCA.sum())
    vidxa_d = nc.dram_tensor("vidxa", [128, ATOT // 16], i16, kind="ExternalInput")
    vidxb_d = nc.dram_tensor("vidxb", [128, IPC * K // 16], i16, kind="ExternalInput")
    relidx_d = nc.dram_tensor("relidx", [128, IPC * K // 16], i16, kind="ExternalInput")
    bu_d = nc.dram_tensor("bu", [128, BC], i32, kind="ExternalInput")
    bp_d = nc.dram_tensor("bp", [128, BC], i32, kind="ExternalInput")
    bn_d = nc.dram_tensor("bn", [128, BC], i32, kind="ExternalInput")
    bw_d = nc.dram_tensor("bw", [128, BC], f32, kind="ExternalInput")
    out_d = nc.dram_tensor("out", [1, 1], f32, kind="ExternalOutput")

    JImax = int(JI.sum(axis=1).max())
    JUmax = int(JU.sum(axis=1).max())

    with tile.TileContext(nc) as tc:
        with (
            tc.tile_pool(name="const", bufs=1) as const,
            tc.tile_pool(name="dram", bufs=1, space="DRAM") as dram,
            tc.tile_pool(name="psum", bufs=1, space="PSUM") as psum,
            tc.tile_pool(name="work", bufs=3) as work,
            tc.tile_pool(name="gath", bufs=2) as gath,
            tc.tile_pool(name="idxp", bufs=3) as idxp,
        ):
            # ---- DRAM scratch (pool tiles => dep-tracked) ----
            vtmp = dram.tile([meta["vtmp_rows"], D], f32, tag="vtmp")
            tl = [
                dram.tile([NR, 128], f32, tag=f"tl{l}", name=f"tl{l}")
                for l in range(2)
            ]
            u1s = dram.tile([UPC, D], f32, tag="u1s")
            u2s = dram.tile([UPC, D], f32, tag="u2s")
            i1s = dram.tile([IPC, D], f32, tag="i1s")
            i2s = dram.tile([IPC, D], f32, tag="i2s")
            u1f = dram.tile([NUP, D], f32, tag="u1f", addr_space="Shared")
            i1f = dram.tile([NIP, D], f32, tag="i1f", addr_space="Shared")
            i2f = dram.tile([NIP, D], f32, tag="i2f", addr_space="Shared")
            ccin = dram.tile([1, 2], f32, tag="ccin")
            ccout = dram.tile([1, 2], f32, tag="ccout")

            # internal copies of gather-source tables (SWDGE gather from
            # ExternalInput appears unsupported under the PJRT path)
            ent_i = dram.tile([NE, D], f32, tag="ent_i")
            nc.sync.dma_start(ent_i[:, :], ent[:, :])
            u0_i = dram.tile([NUP, D], f32, tag="u0_i")
            nc.sync.dma_start(u0_i[:, :], u0[:, :])
            i0_i = dram.tile([NIP, D], f32, tag="i0_i")
            nc.sync.dma_start(i0_i[:, :], i0[:, :])

            ident = const.tile([128, 128], f32, tag="ident")
            make_identity(nc, ident[:])

            # ================= preamble: T_l = [R@Weff | R@b] =================
            rel_sb = const.tile([64, 64], f32, tag="rel_sb")
            nc.sync.dma_start(rel_sb[:], relw[:, :])
            relT_ps = psum.tile([64, 128], f32, tag="t1", bufs=2)
            nc.tensor.transpose(relT_ps[:, :64], rel_sb[:], ident[:64, :64])
            relT = const.tile([64, 64], f32, tag="relT")
            nc.scalar.copy(relT[:], relT_ps[:, :64])

            wkb_sb = const.tile([64, 2], f32, tag="wkb_sb")
            nc.sync.dma_start(wkb_sb[:], wkbT[:, :])
            wab_sb = const.tile([64, 2], f32, tag="wab_sb")
            nc.sync.dma_start(wab_sb[:], wabT[:, :])
            wbb_sb = const.tile([64, 2], f32, tag="wbb_sb")
            nc.sync.dma_start(wbb_sb[:], wbbT[:, :])

            waT_sb = []
            wbT_sb = []
            for l in range(2):
                wa_t = const.tile([64, 64], f32, tag=f"waT{l}", name=f"waT{l}")
                nc.sync.dma_start(wa_t[:], waT[64 * l:64 * l + 64, :])
                waT_sb.append(wa_t)
                wb_t = const.tile([64, 64], f32, tag=f"wbT{l}", name=f"wbT{l}")
                nc.sync.dma_start(wb_t[:], wbT[64 * l:64 * l + 64, :])
                wbT_sb.append(wb_t)
            gbias = const.tile([64, 2], f32, tag="gbias")
            nc.vector.tensor_tensor(gbias[:], wab_sb[:], wbb_sb[:], ALU.add)

            for l in range(2):
                wk_sb = work.tile([64, 128], f32, tag="wk_sb")
                nc.sync.dma_start(wk_sb[:], wk[64 * l:64 * l + 64, :])
                weff = work.tile([64, 64], f32, tag="weff")
                nc.vector.tensor_tensor(
                    weff[:], wk_sb[:, 0:64], wk_sb[:, 64:128], ALU.add
                )
                m_ps = psum.tile([64, 64], f32, tag="t2", bufs=2)
                nc.tensor.matmul(m_ps[:], relT[:], weff[:], start=True, stop=True)
                b_ps = psum.tile([64, 1], f32, tag="gate", bufs=2)
                nc.tensor.matmul(
                    b_ps[:], relT[:], wkb_sb[:, l:l + 1], start=True, stop=True
                )
                t_sb = work.tile([64, 128], f32, tag="t_sb")
                nc.vector.memset(t_sb[:], 0.0)
                nc.scalar.copy(t_sb[:, 0:64], m_ps[:])
                nc.scalar.copy(t_sb[:, 64:65], b_ps[:])
                nc.sync.dma_start(tl[l][:, :], t_sb[:])

            # ================= stage A: compact entity rows into vtmp =========
            for ci in range(NCH if PH >= 1 else 0):
                for ri in range(NRE):
                    n = int(PCA[ci, ri])
                    if n == 0:
                        continue
                    cb = int(runbase[ci, ri])
                    it = idxp.tile([128, n // 16], i16, tag="aidx")
                    nc.sync.dma_start(
                        it[:], vidxa_d[:, cb // 16:(cb + n) // 16]
                    )
                    g = gath.tile([128, n // 128, D], f32, tag="ag")
                    nc.gpsimd.dma_gather(
                        g[:], ent_i[ri * RSE:(ri + 1) * RSE, :], it[:],
                        n, n, D, single_packet=False,
                    )
                    # write to vtmp rows [cb, cb+n): row i = g[i%128, i//128, :]
                    nc.sync.dma_start(
                        vtmp[cb:cb + n, :].rearrange("(c p) d -> p c d", p=128),
                        g[:],
                    )

            # ================= per-layer =================
            kgst = const.tile([128, WI * D], f32, tag="kgst")
            agst = const.tile([128, WI * D], f32, tag="agst")
            icur = const.tile([128, WI * D], f32, tag="icur")
            unext = const.tile([128, WU * D], f32, tag="unext")
            inext = const.tile([128, WI * D], f32, tag="inext")

            for l in range(2 if PH >= 2 else 0):
                utab = u0_i if l == 0 else u1f
                itab = i0_i if l == 0 else i1f
                ish = ishard0 if l == 0 else i1s

                # load own item shard [IPC, D] -> [128, WI*D]
                nc.sync.dma_start(
                    icur[:].rearrange("p (w d) -> p w d", d=D),
                    ish[:, :].rearrange("(w p) d -> p w d", p=128),
                )

                # ---------- KG attention ----------
                for w in range(WI if PH >= 2 else 0):
                    ci = w // VCH
                    cb = int(chbase[ci])
                    ch_n = int(PCA[ci].sum())
                    vit = idxp.tile([128, 128], i16, tag="vidx")
                    nc.sync.dma_start(
                        vit[:], vidxb_d[:, w * 128:(w + 1) * 128]
                    )
                    vt = work.tile([128, K, D], f32, tag="vt")
                    nc.gpsimd.dma_gather(
                        vt[:], vtmp[cb:cb + ch_n, :], vit[:],
                        128 * K, 128 * K, D, single_packet=False,
                    )
                    rit = idxp.tile([128, 128], i16, tag="ridx")
                    nc.sync.dma_start(
                        rit[:], relidx_d[:, w * 128:(w + 1) * 128]
                    )
                    mr = work.tile([128, K, 128], f32, tag="mr")
                    nc.gpsimd.dma_gather(
                        mr[:], tl[l][:, :], rit[:],
                        128 * K, 128 * K, 128, single_packet=False,
                    )
                    ic_w = icur[:, w * D:(w + 1) * D]
                    hv = work.tile([128, K, D], f32, tag="hv")
                    nc.vector.tensor_tensor(
                        hv[:],
                        vt[:],
                        ic_w.unsqueeze(1).to_broadcast([128, K, D]),
                        ALU.mult,
                    )
                    nc.vector.tensor_tensor(
                        hv[:], hv[:], mr[:, :, 0:64], ALU.mult
                    )
                    sc = work.tile([128, K], f32, tag="sc")
                    nc.vector.tensor_reduce(
                        sc[:], hv[:], mybir.AxisListType.X, ALU.add
                    )
                    nc.vector.tensor_tensor(
                        sc[:], sc[:], mr[:, :, 64:65].squeeze(), ALU.add
                    )
                    # leaky relu 0.2
                    sc2 = work.tile([128, K], f32, tag="sc2")
                    nc.vector.tensor_scalar_mul(sc2[:], sc[:], 0.2)
                    nc.vector.tensor_tensor(sc[:], sc[:], sc2[:], ALU.max)
                    # softmax over K
                    mx = work.tile([128, 1], f32, tag="mx")
                    nc.vector.tensor_reduce(
                        mx[:], sc[:], mybir.AxisListType.X, ALU.max, negate=True
                    )
                    nc.scalar.activation(
                        sc[:], sc[:], ACTF.Exp, bias=mx[:, 0:1], scale=1.0
                    )
                    sm = work.tile([128, 1], f32, tag="sm")
                    nc.vector.tensor_reduce(
                        sm[:], sc[:], mybir.AxisListType.X, ALU.add
                    )
                    nc.vector.reciprocal(sm[:], sm[:])
                    nc.vector.tensor_scalar(
                        sc[:], sc[:], sm[:, 0:1], None, ALU.mult
                    )
                    # kg_item = sum_k alpha * v
                    nc.vector.tensor_tensor(
                        vt[:], vt[:],
                        sc[:].unsqueeze(2).to_broadcast([128, K, D]),
                        ALU.mult,
                    )
                    nc.vector.tensor_reduce(
                        kgst[:, w * D:(w + 1) * D],
                        vt[:].rearrange("p k d -> p d k"),
                        mybir.AxisListType.X, ALU.add,
                    )

                # ---------- agg_i ----------
                for w in range(WI if PH >= 3 else 0):
                    sj = int(JI[w].sum())
                    g = gath.tile([128, JImax, D], f32, tag="eg")
                    for r in range(NRU):
                        jj = int(JI[w, r])
                        if jj == 0:
                            continue
                        co = int(cbI[w, r])
                        n = 128 * jj
                        it = idxp.tile([128, n // 16], i16, tag="eidx")
                        nc.sync.dma_start(
                            it[:], eidx_i_d[:, co * 8:(co + jj) * 8]
                        )
                        nc.gpsimd.dma_gather(
                            g[:, co - int(cbI[w, 0]):co - int(cbI[w, 0]) + jj, :],
                            utab[r * RSU:(r + 1) * RSU, :], it[:],
                            n, n, D, single_packet=False,
                        )
                    nt = idxp.tile([128, JImax], f32, tag="enrm")
                    nc.sync.dma_start(
                        nt[:, :sj], enrm_i_d[:, int(cbI[w, 0]):int(cbI[w, 0]) + sj]
                    )
                    nc.vector.tensor_tensor(
                        g[:, :sj, :], g[:, :sj, :],
                        nt[:, :sj].unsqueeze(2).to_broadcast([128, sj, D]),
                        ALU.mult,
                    )
                    nc.vector.tensor_reduce(
                        agst[:, w * D:(w + 1) * D],
                        g[:, :sj, :].rearrange("p j d -> p d j"),
                        mybir.AxisListType.X, ALU.add,
                    )

                # ---------- fusion ----------
                for w in range(WI if PH >= 4 else 0):
                    kg_w = kgst[:, w * D:(w + 1) * D]
                    ag_w = agst[:, w * D:(w + 1) * D]
                    kgT_ps = psum.tile([64, 128], f32, tag="t1", bufs=2)
                    nc.tensor.transpose(kgT_ps[:], kg_w, ident[:])
                    agT_ps = psum.tile([64, 128], f32, tag="t2", bufs=2)
                    nc.tensor.transpose(agT_ps[:], ag_w, ident[:])
                    kgT = work.tile([64, 128], f32, tag="kgT")
                    nc.scalar.copy(kgT[:], kgT_ps[:])
                    agT = work.tile([64, 128], f32, tag="agT")
                    nc.scalar.copy(agT[:], agT_ps[:])
                    gate_ps = psum.tile([64, 128], f32, tag="gate", bufs=2)
                    nc.tensor.matmul(
                        gate_ps[:], waT_sb[l][:], kgT[:],
                        start=True, stop=False,
                    )
                    nc.tensor.matmul(
                        gate_ps[:], wbT_sb[l][:], agT[:],
                        start=False, stop=True,
                    )
                    gate_sb = work.tile([64, 128], f32, tag="gate_sb")
                    nc.scalar.activation(
                        gate_sb[:], gate_ps[:], ACTF.Sigmoid,
                        bias=gbias[:, l:l + 1], scale=1.0,
                    )
                    gT_ps = psum.tile([128, 64], f32, tag="gT", bufs=2)
                    nc.tensor.transpose(gT_ps[:], gate_sb[:], ident[:64, :64])
                    dif = work.tile([128, D], f32, tag="dif")
                    nc.vector.tensor_tensor(dif[:], kg_w, ag_w, ALU.subtract)
                    nc.vector.tensor_tensor(dif[:], dif[:], gT_ps[:], ALU.mult)
                    nc.vector.tensor_tensor(
                        inext[:, w * D:(w + 1) * D], ag_w, dif[:], ALU.add
                    )
                ist = i1s if l == 0 else i2s
                if PH < 4:
                    nc.vector.memset(inext[:], 0.0)
                nc.sync.dma_start(
                    ist[:, :].rearrange("(w p) d -> p w d", p=128),
                    inext[:].rearrange("p (w d) -> p w d", d=D),
                )

                # ---------- agg_u ----------
                for w in range(WU if PH >= 5 else 0):
                    sj = int(JU[w].sum())
                    g = gath.tile([128, JUmax, D], f32, tag="eg")
                    for r in range(NRI):
                        jj = int(JU[w, r])
                        if jj == 0:
                            continue
                        co = int(cbU[w, r])
                        n = 128 * jj
                        it = idxp.tile([128, n // 16], i16, tag="eidx")
                        nc.sync.dma_start(
                            it[:], eidx_u_d[:, co * 8:(co + jj) * 8]
                        )
                        nc.gpsimd.dma_gather(
                            g[:, co - int(cbU[w, 0]):co - int(cbU[w, 0]) + jj, :],
                            itab[r * RSI:(r + 1) * RSI, :], it[:],
                            n, n, D, single_packet=False,
                        )
                    nt = idxp.tile([128, JUmax], f32, tag="enrm")
                    nc.sync.dma_start(
                        nt[:, :sj], enrm_u_d[:, int(cbU[w, 0]):int(cbU[w, 0]) + sj]
                    )
                    nc.vector.tensor_tensor(
                        g[:, :sj, :], g[:, :sj, :],
                        nt[:, :sj].unsqueeze(2).to_broadcast([128, sj, D]),
                        ALU.mult,
                    )
                    nc.vector.tensor_reduce(
                        unext[:, w * D:(w + 1) * D],
                        g[:, :sj, :].rearrange("p j d -> p d j"),
                        mybir.AxisListType.X, ALU.add,
                    )
                ust = u1s if l == 0 else u2s
                if PH < 5:
                    nc.vector.memset(unext[:], 0.0)
                nc.sync.dma_start(
                    ust[:, :].rearrange("(w p) d -> p w d", p=128),
                    unext[:].rearrange("p (w d) -> p w d", d=D),
                )

                # ---------- AllGathers ----------
                rg = [list(range(M))]
                if l == 0:
                    nc.gpsimd.collective_compute(
                        "AllGather", ALU.bypass, replica_groups=rg,
                        ins=[u1s.opt()], outs=[u1f.opt()],
                    )
                    nc.gpsimd.collective_compute(
                        "AllGather", ALU.bypass, replica_groups=rg,
                        ins=[i1s.opt()], outs=[i1f.opt()],
                    )
                else:
                    nc.gpsimd.collective_compute(
                        "AllGather", ALU.bypass, replica_groups=rg,
                        ins=[i2s.opt()], outs=[i2f.opt()],
                    )

            # ================= BPR =================
            import concourse.bass as bass_mod

            bu_sb = const.tile([128, BC], i32, tag="bu_sb")
            nc.sync.dma_start(bu_sb[:], bu_d[:, :])
            bp_sb = const.tile([128, BC], i32, tag="bp_sb")
            nc.sync.dma_start(bp_sb[:], bp_d[:, :])
            bn_sb = const.tile([128, BC], i32, tag="bn_sb")
            nc.sync.dma_start(bn_sb[:], bn_d[:, :])
            bw_sb = const.tile([128, BC], f32, tag="bw_sb")
            nc.sync.dma_start(bw_sb[:], bw_d[:, :])

            ue = const.tile([128, BC, D], f32, tag="ue")
            pe = const.tile([128, BC, D], f32, tag="pe")
            ne_ = const.tile([128, BC, D], f32, tag="ne")
            tmp = work.tile([128, BC, D], f32, tag="btmp")

            def gather3(dst, idx_sb, tabs):
                first = True
                for tab in tabs:
                    for cc in range(BC):
                        tgt = dst[:, cc, :] if first else tmp[:, cc, :]
                        nc.gpsimd.indirect_dma_start(
                            out=tgt,
                            out_offset=None,
                            in_=tab[:, :],
                            in_offset=bass_mod.IndirectOffsetOnAxis(
                                ap=idx_sb[:, cc:cc + 1], axis=0
                            ),
                        )
                    if not first:
                        nc.vector.tensor_tensor(dst[:], dst[:], tmp[:], ALU.add)
                    first = False

            gather3(ue, bu_sb, [ushard0, u1s, u2s])
            gather3(pe, bp_sb, [i0, i1f, i2f])
            gather3(ne_, bn_sb, [i0, i1f, i2f])

            # l2 partial: sum bw * (ue^2+pe^2+ne^2) -- but weights apply per
            # entry; squares summed over d
            sq = work.tile([128, BC], f32, tag="sq")
            l2acc = work.tile([128, BC], f32, tag="l2acc")
            ps = work.tile([128, BC], f32, tag="ps")
            ns = work.tile([128, BC], f32, tag="ns")

            nc.vector.tensor_tensor(tmp[:], ue[:], ue[:], ALU.mult)
            nc.vector.tensor_reduce(
                l2acc[:], tmp[:], mybir.AxisListType.X, ALU.add
            )
            nc.vector.tensor_tensor(tmp[:], pe[:], pe[:], ALU.mult)
            nc.vector.tensor_reduce(sq[:], tmp[:], mybir.AxisListType.X, ALU.add)
            nc.vector.tensor_tensor(l2acc[:], l2acc[:], sq[:], ALU.add)
            nc.vector.tensor_tensor(tmp[:], ne_[:], ne_[:], ALU.mult)
            nc.vector.tensor_reduce(sq[:], tmp[:], mybir.AxisListType.X, ALU.add)
            nc.vector.tensor_tensor(l2acc[:], l2acc[:], sq[:], ALU.add)
            nc.vector.tensor_tensor(l2acc[:], l2acc[:], bw_sb[:], ALU.mult)

            nc.vector.tensor_tensor(tmp[:], ue[:], pe[:], ALU.mult)
            nc.vector.tensor_reduce(ps[:], tmp[:], mybir.AxisListType.X, ALU.add)
            nc.vector.tensor_tensor(tmp[:], ue[:], ne_[:], ALU.mult)
            nc.vector.tensor_reduce(ns[:], tmp[:], mybir.AxisListType.X, ALU.add)
            nc.vector.tensor_tensor(ps[:], ps[:], ns[:], ALU.subtract)
            # log(sigmoid(x) + 1e-10)
            eps_t = const.tile([128, 1], f32, tag="eps_t")
            nc.vector.memset(eps_t[:], 1e-10)
            nc.scalar.activation(ps[:], ps[:], ACTF.Sigmoid)
            nc.scalar.activation(
                ps[:], ps[:], ACTF.Ln, bias=eps_t[:, 0:1], scale=1.0
            )
            nc.vector.tensor_tensor(ps[:], ps[:], bw_sb[:], ALU.mult)

            # partials: reduce to scalars
            part = work.tile([128, 2], f32, tag="part")
            nc.vector.tensor_reduce(
                part[:, 0:1], ps[:], mybir.AxisListType.X, ALU.add
            )
            nc.vector.tensor_reduce(
                part[:, 1:2], l2acc[:], mybir.AxisListType.X, ALU.add
            )
            import concourse.bass_isa as bass_isa
            partr = work.tile([128, 2], f32, tag="partr")
            nc.gpsimd.partition_all_reduce(
                partr[:], part[:], 128, bass_isa.ReduceOp.add
            )
            # AllReduce across cores
            nc.sync.dma_start(ccin[:, :], partr[0:1, :])
            nc.gpsimd.collective_compute(
                "AllReduce", ALU.add, replica_groups=[list(range(M))],
                ins=[ccin.opt()], outs=[ccout.opt()],
            )
            fin = work.tile([1, 2], f32, tag="fin")
            nc.sync.dma_start(fin[:], ccout[:, :])
            # out = -slog/B + REG*l2/B
            res = work.tile([1, 1], f32, tag="res")
            nc.vector.tensor_scalar_mul(res[:], fin[:, 0:1], -1.0 / B)
            res2 = work.tile([1, 1], f32, tag="res2")
            nc.vector.tensor_scalar_mul(res2[:], fin[:, 1:2], REG / B)
            nc.vector.tensor_tensor(res[:], res[:], res2[:], ALU.add)
            nc.sync.dma_start(out_d[:, :], res[:])

    nc.compile()
    return nc


LAST_EXEC_NS = None


def _run_timed(nc, in_maps, n_cores, iters=6):
    """Replicates bass2jax.run_bass_via_pjrt's multi-core path but keeps the
    jitted executable and times repeated steady-state executions."""
    import time
    import jax
    import numpy as _np
    from jax.sharding import Mesh, PartitionSpec
    from jax.experimental.shard_map import shard_map
    import concourse.mybir as mybir
    from concourse import bass2jax as B2J

    B2J.install_neuronx_cc_hook()
    in_names, out_names, out_avals, zero_outs = [], [], [], []
    partition_name = (
        nc.partition_id_tensor.name if nc.partition_id_tensor else None
    )
    for alloc in nc.m.functions[0].allocations:
        if not isinstance(alloc, mybir.MemoryLocationSet):
            continue
        name = alloc.memorylocations[0].name
        if alloc.kind == "ExternalInput":
            if name != partition_name:
                in_names.append(name)
        elif alloc.kind == "ExternalOutput":
            out_names.append(name)
            shape = tuple(alloc.tensor_shape)
            dtype = mybir.dt.np(alloc.dtype)
            out_avals.append(jax.core.ShapedArray(shape, dtype))
            zero_outs.append(_np.zeros(shape, dtype))
    n_params = len(in_names)
    n_outs = len(out_avals)
    in_names.extend(out_names)
    if partition_name is not None:
        in_names.append(partition_name)
    donate = tuple(range(n_params, n_params + n_outs))

    def _body(*args):
        operands = list(args)
        if partition_name is not None:
            operands.append(B2J.partition_id_tensor())
        outs = B2J._bass_exec_p.bind(
            *operands,
            out_avals=tuple(out_avals),
            in_names=tuple(in_names),
            out_names=tuple(out_names),
            lowering_input_output_aliases=(),
            sim_require_finite=True,
            sim_require_nnan=True,
            nc=nc,
        )
        return tuple(outs)

    devices = jax.devices()[:n_cores]
    mesh = Mesh(_np.asarray(devices), ("core",))
    in_specs = (PartitionSpec("core"),) * (n_params + n_outs)
    out_specs = (PartitionSpec("core"),) * len(out_names)
    sharded = jax.jit(
        shard_map(_body, mesh=mesh, in_specs=in_specs, out_specs=out_specs,
                  check_rep=False),
        donate_argnums=donate, keep_unused=True,
    )
    sh = jax.sharding.NamedSharding(mesh, PartitionSpec("core"))
    concat_in = [
        jax.device_put(
            _np.concatenate(
                [_np.asarray(in_maps[c][in_names[i]]) for c in range(n_cores)],
                axis=0,
            ),
            sh,
        )
        for i in range(n_params)
    ]
    times = []
    outs = None
    for _ in range(iters):
        concat_zeros = [
            jax.device_put(
                _np.zeros((n_cores * z.shape[0], *z.shape[1:]), z.dtype), sh
            )
            for z in zero_outs
        ]
        jax.block_until_ready(concat_zeros)
        jax.block_until_ready(concat_in)
        t0 = time.perf_counter()
        outs = sharded(*concat_in, *concat_zeros)
        jax.block_until_ready(outs)
        times.append(time.perf_counter() - t0)
    res = [
        {
            name: _np.asarray(outs[i]).reshape(n_cores, *out_avals[i].shape)[c]
            for i, name in enumerate(out_names)
        }
        for c in range(n_cores)
    ]
    return res, times


def kernel(**inputs):
    global LAST_EXEC_NS
    import os
    meta, in_maps = _prep(inputs)
    nc = _build(meta)
    from concourse.bass_utils import run_bass_kernel_spmd

    if bool(int(os.environ.get("AKDN_TIME", "0"))):
        results, times = _run_timed(nc, in_maps, M)
        LAST_EXEC_NS = int(min(times) * 1e9)
        print("exec times (s):", [f"{t:.5f}" for t in times])
        out = results[0]["out"]
        return np.float32(out.reshape(())[()])
    res = run_bass_kernel_spmd(nc, in_maps, core_ids=list(range(M)))
    LAST_EXEC_NS = res.exec_time_ns
    out = res.results[0]["out"]
    return np.float32(out.reshape(())[()])
